# revision 18
# baseline (speedup 1.0000x reference)
"""Trainium2 Bass kernel for nn_DCConv3dKernelPolynomials.

out[o,i,x,n] = sum_b basis_b(position[x,n]) * coeffs[o,i,b]

Strategy (per the sharding hint): shard the 110592 grid points across the 8
NeuronCores (13824 each), replicate the folded coefficient matrix.  Per core:
 - evaluate the 30 hydrogen-wavefunction basis functions point-major on
   DVE/ACT (all scalar normalizations folded into the coefficients host-side,
   Laguerre polynomials factored into real linear roots),
 - PE-transpose psi into a (30 x points) layout, 4 point-groups packed into
   the 128 partitions,
 - row-tiled fp32r matmuls (K=30 per 32-row group) against the replicated
   coefficients -> PSUM, evacuate via DVE/ACT to SBUF, DMA out.
"""
import math

import numpy as np

OUTC, INC = 16, 16
OUTN, CONV_N = 4096, 27
NB = 30
NCORES = 8
PTS = OUTN * CONV_N            # 110592
CPTS = PTS // NCORES           # 13824 per core
NGRP = 4                       # point groups (matmul row tiling)
GPTS = CPTS // NGRP            # 3456 per group
NROUND = GPTS // 128           # 27 transpose rounds
NT = 7                         # output column chunks per group (6x512 + 384)
PI = math.pi


# ----------------------------------------------------------------- constants
def _qnums():
    lst = []
    for n in range(1, 5):
        for l in range(0, min(n, 4)):
            for m in range(-l, l + 1):
                if abs(m) <= 3:
                    lst.append((n, l, m))
    return lst


QNUMS = _qnums()


def _laguerre_coeffs(k, alpha):
    return [((-1.0) ** i) * math.comb(k + alpha, k - i) / math.factorial(i)
            for i in range(k + 1)]


def _radial_info(n, l):
    k = n - l - 1
    lag = _laguerre_coeffs(k, 2 * l + 1)
    cr = [lag[i] * (2.0 / n) ** i for i in range(k + 1)]
    norm_r = math.sqrt((2.0 / n) ** 3 * math.factorial(n - l - 1)
                       / (2.0 * n * math.factorial(n + l)))
    lead = cr[-1]
    K_rad = norm_r * (2.0 / n) ** l * lead
    roots = [] if k == 0 else sorted(float(x) for x in
                                     np.real(np.roots(np.array(cr[::-1]))))
    return roots, K_rad


_K_ANG = {(0, 0): 1.0, (1, 0): 1.0, (1, 1): -1.0,
          (2, 0): 1.5, (2, 1): -3.0, (2, 2): 3.0,
          (3, 0): 2.5, (3, 1): -7.5, (3, 2): 15.0, (3, 3): -15.0}
_TRIGFOLD = {0: 1.0, 1: 1.0, -1: 1.0, 2: 2.0, -2: 2.0, 3: 4.0, -3: 4.0}

ROOTS20 = _radial_info(2, 0)[0]
ROOTS30 = _radial_info(3, 0)[0]
ROOTS31 = _radial_info(3, 1)[0]
ROOTS40 = _radial_info(4, 0)[0]
ROOTS41 = _radial_info(4, 1)[0]
ROOTS42 = _radial_info(4, 2)[0]


def _fold_constants():
    K = np.zeros(NB)
    for b, (n, l, m) in enumerate(QNUMS):
        am = abs(m)
        _, K_rad = _radial_info(n, l)
        klm = math.sqrt((2.0 * l + 1.0) / (4.0 * PI)
                        * math.factorial(l - am) / math.factorial(l + am))
        K[b] = (K_rad * klm * (math.sqrt(2.0) if m != 0 else 1.0)
                * _K_ANG[(l, am)] * _TRIGFOLD[m])
    return K


# ------------------------------------------------------------- device program
_PROGRAM_CACHE = {}


def _build_program():
    import concourse.bacc as bacc
    import concourse.tile as tile
    from concourse import mybir

    f32 = mybir.dt.float32
    f32r = mybir.dt.float32r
    AF = mybir.ActivationFunctionType

    nc = bacc.Bacc("TRN2", debug=False, num_devices=NCORES)

    # activation() lowers float biases via pre-registered const APs; add ours
    # the same way Bass.__init__ registers 0.0/1.0.
    for val in (PI / 2, PI):
        t = nc.alloc_sbuf_tensor(f"const-float32-{val}", [128, 1], f32)
        nc.gpsimd.memset(t.ap(), val)
        nc.const_aps.aps[(f32, val)] = t.ap()
    nc.all_engine_barrier()

    pos_d = nc.dram_tensor("pos", [3, 128, NROUND * NGRP], f32,
                           kind="ExternalInput")
    wts_d = nc.dram_tensor("wts", [128, 256], f32, kind="ExternalInput")
    ident_d = nc.dram_tensor("ident", [128, 128], f32, kind="ExternalInput")
    out_d = nc.dram_tensor("out", [256, CPTS], mybir.dt.bfloat16,
                           kind="ExternalOutput")

    with tile.TileContext(nc) as tc:
        _kernel_body(tc, nc, out_d.ap(), pos_d.ap(), wts_d.ap(), ident_d.ap(),
                     f32, f32r, AF)
    nc.compile()
    return nc


def _kernel_body(tc, nc, out_ap, pos_ap, wts_ap, ident_ap, f32, f32r, AF):
    from contextlib import ExitStack
    from concourse import mybir
    Alu = mybir.AluOpType

    ctx = ExitStack()
    with ctx:
        const = ctx.enter_context(tc.tile_pool(name="const", bufs=1))
        feat = ctx.enter_context(tc.tile_pool(name="feat", bufs=1))
        pT = ctx.enter_context(tc.tile_pool(name="pT", bufs=2, space="PSUM"))
        pM = ctx.enter_context(tc.tile_pool(name="pM", bufs=3, space="PSUM"))
        stg = ctx.enter_context(tc.tile_pool(name="stg", bufs=8))

        F = NROUND * NGRP       # 108 g-columns (g = c*4 + j)
        # pipeline segments: g-range -> t-chunks it covers (t needs g[16t:16t+16))
        SEGS = [(0, 16, [0]), (16, 48, [1, 2]), (48, F, [3, 4, 5, 6])]

        def ft(name):
            t = feat.tile([128, F], f32, tag=name)
            return t

        r = ft("r"); th = ft("th"); ph = ft("ph")
        nc.sync.dma_start(th[:], pos_ap[1])
        nc.sync.dma_start(ph[:], pos_ap[2])
        nc.sync.dma_start(r[:], pos_ap[0])
        wts = const.tile([128, 256], f32)
        nc.sync.dma_start(wts[:], wts_ap)
        ident = const.tile([128, 128], f32)
        nc.sync.dma_start(ident[:], ident_ap)
        wtsr = const.tile([128, 256], f32r)
        nc.scalar.copy(wtsr[:], wts[:])

        # psi point-major, one tile per segment: PMs[i][p, g-g0, bb]
        PMs = []
        for i, (g0, g1, _) in enumerate(SEGS):
            pm = feat.tile([128, g1 - g0, 32], f32, tag=f"PM{i}")
            nc.vector.memset(pm[:, :, NB:32], 0.0)
            PMs.append(pm)

        act = nc.scalar.activation
        stt = nc.vector.scalar_tensor_tensor
        tt = nc.vector.tensor_tensor
        ts = nc.vector.tensor_scalar
        gtt = nc.gpsimd.tensor_tensor

        # ---- seeds (ACT: Sin set, then Exp set; squares on DVE) ----
        sth = ft("sth"); u = ft("u"); s1 = ft("s1"); sh = ft("sh")
        act(sth[:], th[:], AF.Sin)                          # sin(theta)
        act(u[:], th[:], AF.Sin, scale=-1.0, bias=PI / 2)   # cos(theta)
        act(s1[:], ph[:], AF.Sin, scale=-1.0, bias=PI)      # sin(phi)
        act(sh[:], ph[:], AF.Sin, scale=0.5)                # sin(phi/2)
        E2 = ft("E2"); E3 = ft("E3"); E4 = ft("E4")
        act(E2[:], r[:], AF.Exp, scale=-0.5)
        act(E3[:], r[:], AF.Exp, scale=-1.0 / 3.0)
        act(E4[:], r[:], AF.Exp, scale=-0.25)
        shsq = ft("shsq"); u2 = ft("u2"); stsq = ft("stsq")
        tt(shsq[:], sh[:], sh[:], Alu.mult)
        tt(u2[:], u[:], u[:], Alu.mult)
        tt(stsq[:], sth[:], sth[:], Alu.mult)

        # ---- trig ladders / angular (late-consumer ops on GPSIMD) ----
        c1 = ft("c1")
        ts(c1[:], shsq[:], -2.0, 1.0, Alu.mult, Alu.add)    # cos(phi)
        c1sq = ft("c1sq")
        tt(c1sq[:], c1[:], c1[:], Alu.mult)
        c2t = ft("c2t")
        ts(c2t[:], c1sq[:], -0.5, None, Alu.add)            # cos(2phi)/2
        s2t = ft("s2t")
        tt(s2t[:], s1[:], c1[:], Alu.mult)                  # sin(2phi)/2
        c3t = ft("c3t")
        stt(c3t[:], c1sq[:], 0.75, c1[:], Alu.subtract, Alu.mult)   # cos3/4
        s3t = ft("s3t")
        stt(s3t[:], c1sq[:], 0.25, s1[:], Alu.subtract, Alu.mult)   # sin3/4
        p20 = ft("p20")
        ts(p20[:], u2[:], -1.0 / 3.0, None, Alu.add)
        p30 = ft("p30")
        stt(p30[:], u2[:], 0.6, u[:], Alu.subtract, Alu.mult)
        p33 = ft("p33")
        gtt(p33[:], sth[:], stsq[:], Alu.mult)              # sin^3(theta)
        A1c = ft("A1c"); A1s = ft("A1s")
        tt(A1c[:], sth[:], c1[:], Alu.mult)
        tt(A1s[:], sth[:], s1[:], Alu.mult)
        A2c1 = ft("A2c1"); A2s1 = ft("A2s1")
        tt(A2c1[:], u[:], A1c[:], Alu.mult)
        tt(A2s1[:], u[:], A1s[:], Alu.mult)
        A2c2 = ft("A2c2"); A2s2 = ft("A2s2")
        tt(A2c2[:], stsq[:], c2t[:], Alu.mult)
        tt(A2s2[:], stsq[:], s2t[:], Alu.mult)
        A3c1 = ft("A3c1"); A3s1 = ft("A3s1")
        stt(A3c1[:], u2[:], 0.2, A1c[:], Alu.subtract, Alu.mult)
        stt(A3s1[:], u2[:], 0.2, A1s[:], Alu.subtract, Alu.mult)
        A3c2 = ft("A3c2"); A3s2 = ft("A3s2")
        gtt(A3c2[:], u[:], A2c2[:], Alu.mult)
        gtt(A3s2[:], u[:], A2s2[:], Alu.mult)
        A3c3 = ft("A3c3"); A3s3 = ft("A3s3")
        gtt(A3c3[:], p33[:], c3t[:], Alu.mult)
        gtt(A3s3[:], p33[:], s3t[:], Alu.mult)

        # ---- radial (TT sub-chain on GPSIMD, stt stays on DVE) ----
        E2r = ft("E2r"); E3r = ft("E3r"); E4r = ft("E4r")
        gtt(E2r[:], E2[:], r[:], Alu.mult)
        gtt(E3r[:], E3[:], r[:], Alu.mult)
        gtt(E4r[:], E4[:], r[:], Alu.mult)
        R31 = ft("R31")
        stt(R31[:], r[:], ROOTS31[0], E3r[:], Alu.subtract, Alu.mult)
        R32 = ft("R32")
        gtt(R32[:], E3r[:], r[:], Alu.mult)
        E4r2 = ft("E4r2")
        gtt(E4r2[:], E4r[:], r[:], Alu.mult)
        R41a = ft("R41a"); R41 = ft("R41")
        stt(R41a[:], r[:], ROOTS41[0], E4r[:], Alu.subtract, Alu.mult)
        stt(R41[:], r[:], ROOTS41[1], R41a[:], Alu.subtract, Alu.mult)
        R42 = ft("R42")
        stt(R42[:], r[:], ROOTS42[0], E4r2[:], Alu.subtract, Alu.mult)
        R43 = ft("R43")
        gtt(R43[:], E4r2[:], r[:], Alu.mult)
        t35 = ft("t35")
        stt(t35[:], r[:], ROOTS30[0], E3[:], Alu.subtract, Alu.mult)
        t41 = ft("t41"); t42 = ft("t42")
        stt(t41[:], r[:], ROOTS40[0], E4[:], Alu.subtract, Alu.mult)
        stt(t42[:], r[:], ROOTS40[1], t41[:], Alu.subtract, Alu.mult)

        # ---- pipelined: psi seg-products, transposes, matmuls, DMA ----
        psiT = feat.tile([128, GPTS], f32r)
        bf16 = mybir.dt.bfloat16
        out3 = out_ap.rearrange("o (j p) -> o j p", j=NGRP)

        def psi_seg(si):
            g0, g1, _ = SEGS[si]
            PM = PMs[si]
            sl = slice(g0, g1)

            def pslot(b):
                return PM[:, :, b]

            act(pslot(0), r[:, sl], AF.Exp, scale=-1.0)
            stt(pslot(1), r[:, sl], ROOTS20[0], E2[:, sl],
                Alu.subtract, Alu.mult)
            tt(pslot(2), E2r[:, sl], A1s[:, sl], Alu.mult)
            tt(pslot(3), E2r[:, sl], u[:, sl], Alu.mult)
            tt(pslot(4), E2r[:, sl], A1c[:, sl], Alu.mult)
            stt(pslot(5), r[:, sl], ROOTS30[1], t35[:, sl],
                Alu.subtract, Alu.mult)
            tt(pslot(6), R31[:, sl], A1s[:, sl], Alu.mult)
            tt(pslot(7), R31[:, sl], u[:, sl], Alu.mult)
            tt(pslot(8), R31[:, sl], A1c[:, sl], Alu.mult)
            gtt(pslot(9), R32[:, sl], A2s2[:, sl], Alu.mult)
            tt(pslot(10), R32[:, sl], A2s1[:, sl], Alu.mult)
            tt(pslot(11), R32[:, sl], p20[:, sl], Alu.mult)
            tt(pslot(12), R32[:, sl], A2c1[:, sl], Alu.mult)
            gtt(pslot(13), R32[:, sl], A2c2[:, sl], Alu.mult)
            stt(pslot(14), r[:, sl], ROOTS40[2], t42[:, sl],
                Alu.subtract, Alu.mult)
            tt(pslot(15), R41[:, sl], A1s[:, sl], Alu.mult)
            tt(pslot(16), R41[:, sl], u[:, sl], Alu.mult)
            tt(pslot(17), R41[:, sl], A1c[:, sl], Alu.mult)
            gtt(pslot(18), R42[:, sl], A2s2[:, sl], Alu.mult)
            tt(pslot(19), R42[:, sl], A2s1[:, sl], Alu.mult)
            tt(pslot(20), R42[:, sl], p20[:, sl], Alu.mult)
            tt(pslot(21), R42[:, sl], A2c1[:, sl], Alu.mult)
            gtt(pslot(22), R42[:, sl], A2c2[:, sl], Alu.mult)
            gtt(pslot(23), R43[:, sl], A3s3[:, sl], Alu.mult)
            gtt(pslot(24), R43[:, sl], A3s2[:, sl], Alu.mult)
            tt(pslot(25), R43[:, sl], A3s1[:, sl], Alu.mult)
            tt(pslot(26), R43[:, sl], p30[:, sl], Alu.mult)
            tt(pslot(27), R43[:, sl], A3c1[:, sl], Alu.mult)
            gtt(pslot(28), R43[:, sl], A3c2[:, sl], Alu.mult)
            gtt(pslot(29), R43[:, sl], A3c3[:, sl], Alu.mult)

        def transposes_seg(si):
            g0, g1, _ = SEGS[si]
            PM = PMs[si]
            c_lo, c_hi = g0 // 4, g1 // 4
            for cb in range(c_lo, c_hi, 4):
                nb4 = min(4, c_hi - cb)
                tp = pT.tile([128, 512], f32, tag="tp")
                for ci in range(nb4):
                    c = cb + ci
                    nc.tensor.transpose(
                        tp[:, ci * 128:(ci + 1) * 128],
                        PM[:, 4 * c - g0:4 * c - g0 + 4, :], ident[:])
                dst = psiT[:, cb * 128:(cb + nb4) * 128]
                if (cb // 4) % 2 == 0:
                    nc.scalar.copy(dst, tp[:, :nb4 * 128])
                else:
                    nc.vector.tensor_copy(dst, tp[:, :nb4 * 128])

        evac_state = [0]

        def mm_chunk(t, dve_evac_mod):
            n = min(512, GPTS - t * 512)
            for h in range(2):
                so = stg.tile([128, 2048], bf16, tag="so")
                so4 = so.rearrange("p (j q) -> p j q", q=512)
                for jp in (0, 2):
                    ps = pM.tile([128, 1024], f32, tag="ps")
                    for jj in (0, 1):
                        j = jp + jj
                        lhsT = wtsr[32 * j:32 * j + NB,
                                    128 * h:128 * (h + 1)]
                        rhs = psiT[32 * j:32 * j + NB, t * 512:t * 512 + n]
                        nc.tensor.matmul(ps[:, jj * 512:jj * 512 + n],
                                         lhsT, rhs, start=True, stop=True,
                                         tile_position=(32 * j, 0))
                    # one wide copy covers both j outputs; the [n:512] gap
                    # is never DMA'd so copying it is harmless
                    w = 512 + n
                    i = evac_state[0]
                    evac_state[0] += 1
                    if i % 3 == 0:
                        nc.vector.tensor_copy(
                            so[:, jp * 512:jp * 512 + w], ps[:, :w])
                    else:
                        nc.scalar.copy(
                            so[:, jp * 512:jp * 512 + w], ps[:, :w])
                dst = out3[128 * h:128 * (h + 1), :, t * 512:t * 512 + n]
                nc.sync.dma_start(dst, so4[:, :, :n])

        # emission order = scheduler priority: psi products of the next
        # segment outrank evacuations of the previous one on DVE
        psi_seg(0)
        transposes_seg(0)
        psi_seg(1)
        mm_chunk(0, 0)          # early evacs: ACT only
        transposes_seg(1)
        psi_seg(2)
        mm_chunk(1, 0)
        mm_chunk(2, 0)
        transposes_seg(2)
        for t in (3, 4, 5, 6):
            mm_chunk(t, 2)      # late evacs: alternate DVE/ACT


def _get_program():
    if "nc" not in _PROGRAM_CACHE:
        _PROGRAM_CACHE["nc"] = _build_program()
    return _PROGRAM_CACHE["nc"]


# ---------------------------------------------------------------- host wrapper
def _host_prep(position, coeffs):
    K = _fold_constants()
    Cs = (np.asarray(coeffs, np.float64).reshape(OUTC * INC, NB)
          * K[None, :]).astype(np.float32)
    W = np.zeros((128, 256), np.float32)
    for j in range(NGRP):
        W[32 * j:32 * j + NB, :] = Cs.T
    pts = np.asarray(position, np.float32).reshape(PTS, 3)
    pos_cores = []
    for k in range(NCORES):
        sl = pts[k * CPTS:(k + 1) * CPTS]
        v = sl.reshape(NGRP, NROUND, 128, 3)       # [j, c, p, coord]
        v = np.transpose(v, (3, 2, 1, 0))          # [coord, p, c, j]
        pos_cores.append(np.ascontiguousarray(v.reshape(3, 128, NROUND * NGRP)))
    return pos_cores, W


def kernel(position, coeffs, _collect=None):
    from concourse.bass_utils import run_bass_kernel_spmd

    pos_cores, W = _host_prep(position, coeffs)
    ident = np.eye(128, dtype=np.float32)
    in_maps = [{"pos": pos_cores[k], "wts": W, "ident": ident}
               for k in range(NCORES)]
    nc = _get_program()
    try:
        res = run_bass_kernel_spmd(nc, in_maps, core_ids=list(range(NCORES)))
    except Exception:
        # transient NRT/axon failures (e.g. a wedged core from a previous
        # process) usually clear on retry
        res = run_bass_kernel_spmd(nc, in_maps, core_ids=list(range(NCORES)))
    if _collect is not None:
        _collect.append(res)
    full = np.concatenate(
        [np.asarray(res.results[k]["out"]).astype(np.float32)
         for k in range(NCORES)], axis=1)
    return full.reshape(OUTC, INC, OUTN, CONV_N)


# revision 24
# speedup vs baseline: 1.0258x; 1.0258x over previous
"""Trainium2 Bass kernel for nn_DCConv3dKernelPolynomials.

out[o,i,x,n] = sum_b basis_b(position[x,n]) * coeffs[o,i,b]

Strategy (per the sharding hint): shard the 110592 grid points across the 8
NeuronCores (13824 each), replicate the folded coefficient matrix.  Per core:
 - evaluate the 30 hydrogen-wavefunction basis functions point-major on
   DVE/ACT (all scalar normalizations folded into the coefficients host-side,
   Laguerre polynomials factored into real linear roots),
 - PE-transpose psi into a (30 x points) layout, 4 point-groups packed into
   the 128 partitions,
 - row-tiled fp32r matmuls (K=30 per 32-row group) against the replicated
   coefficients -> PSUM, evacuate via DVE/ACT to SBUF, DMA out.
"""
import math

import numpy as np

OUTC, INC = 16, 16
OUTN, CONV_N = 4096, 27
NB = 30
NCORES = 8
PTS = OUTN * CONV_N            # 110592
CPTS = PTS // NCORES           # 13824 per core
NGRP = 4                       # point groups (matmul row tiling)
GPTS = CPTS // NGRP            # 3456 per group
NROUND = GPTS // 128           # 27 transpose rounds
NT = 7                         # output column chunks per group (6x512 + 384)
PI = math.pi


# ----------------------------------------------------------------- constants
def _qnums():
    lst = []
    for n in range(1, 5):
        for l in range(0, min(n, 4)):
            for m in range(-l, l + 1):
                if abs(m) <= 3:
                    lst.append((n, l, m))
    return lst


QNUMS = _qnums()


def _laguerre_coeffs(k, alpha):
    return [((-1.0) ** i) * math.comb(k + alpha, k - i) / math.factorial(i)
            for i in range(k + 1)]


def _radial_info(n, l):
    k = n - l - 1
    lag = _laguerre_coeffs(k, 2 * l + 1)
    cr = [lag[i] * (2.0 / n) ** i for i in range(k + 1)]
    norm_r = math.sqrt((2.0 / n) ** 3 * math.factorial(n - l - 1)
                       / (2.0 * n * math.factorial(n + l)))
    lead = cr[-1]
    K_rad = norm_r * (2.0 / n) ** l * lead
    roots = [] if k == 0 else sorted(float(x) for x in
                                     np.real(np.roots(np.array(cr[::-1]))))
    return roots, K_rad


_K_ANG = {(0, 0): 1.0, (1, 0): 1.0, (1, 1): -1.0,
          (2, 0): 1.5, (2, 1): -3.0, (2, 2): 3.0,
          (3, 0): 2.5, (3, 1): -7.5, (3, 2): 15.0, (3, 3): -15.0}
_TRIGFOLD = {0: 1.0, 1: 1.0, -1: 1.0, 2: 2.0, -2: 2.0, 3: 4.0, -3: 4.0}

ROOTS20 = _radial_info(2, 0)[0]
ROOTS30 = _radial_info(3, 0)[0]
ROOTS31 = _radial_info(3, 1)[0]
ROOTS40 = _radial_info(4, 0)[0]
ROOTS41 = _radial_info(4, 1)[0]
ROOTS42 = _radial_info(4, 2)[0]


def _fold_constants():
    K = np.zeros(NB)
    for b, (n, l, m) in enumerate(QNUMS):
        am = abs(m)
        _, K_rad = _radial_info(n, l)
        klm = math.sqrt((2.0 * l + 1.0) / (4.0 * PI)
                        * math.factorial(l - am) / math.factorial(l + am))
        K[b] = (K_rad * klm * (math.sqrt(2.0) if m != 0 else 1.0)
                * _K_ANG[(l, am)] * _TRIGFOLD[m])
    return K


# ------------------------------------------------------------- device program
_PROGRAM_CACHE = {}


def _build_program():
    import concourse.bacc as bacc
    import concourse.tile as tile
    from concourse import mybir

    f32 = mybir.dt.float32
    f32r = mybir.dt.float32r
    AF = mybir.ActivationFunctionType

    nc = bacc.Bacc("TRN2", debug=False, num_devices=NCORES)

    # activation() lowers float biases via pre-registered const APs; add ours
    # the same way Bass.__init__ registers 0.0/1.0.
    for val in (PI / 2, PI):
        t = nc.alloc_sbuf_tensor(f"const-float32-{val}", [128, 1], f32)
        nc.gpsimd.memset(t.ap(), val)
        nc.const_aps.aps[(f32, val)] = t.ap()
    nc.all_engine_barrier()

    pos_d = nc.dram_tensor("pos", [3, 128, NROUND * NGRP], f32,
                           kind="ExternalInput")
    wts_d = nc.dram_tensor("wts", [128, 256], f32, kind="ExternalInput")
    ident_d = nc.dram_tensor("ident", [128, 128], f32, kind="ExternalInput")
    out_d = nc.dram_tensor("out", [256, CPTS], mybir.dt.bfloat16,
                           kind="ExternalOutput")

    with tile.TileContext(nc) as tc:
        _kernel_body(tc, nc, out_d.ap(), pos_d.ap(), wts_d.ap(), ident_d.ap(),
                     f32, f32r, AF)
    nc.compile()
    return nc


def _kernel_body(tc, nc, out_ap, pos_ap, wts_ap, ident_ap, f32, f32r, AF):
    from contextlib import ExitStack
    from concourse import mybir
    Alu = mybir.AluOpType

    ctx = ExitStack()
    with ctx:
        const = ctx.enter_context(tc.tile_pool(name="const", bufs=1))
        feat = ctx.enter_context(tc.tile_pool(name="feat", bufs=1))
        pT = ctx.enter_context(tc.tile_pool(name="pT", bufs=2, space="PSUM"))
        pM = ctx.enter_context(tc.tile_pool(name="pM", bufs=3, space="PSUM"))
        stg = ctx.enter_context(tc.tile_pool(name="stg", bufs=8))

        bf16 = mybir.dt.bfloat16
        F = NROUND * NGRP       # 108 g-columns (g = c*4 + j)
        # pipeline segments: g-range -> t-chunks it covers (t needs g[16t:16t+16))
        SEGS = [(0, 16, [0]), (16, 48, [1, 2]), (48, F, [3, 4, 5, 6])]

        def ft(name):
            t = feat.tile([128, F], f32, tag=name)
            return t

        posT = feat.tile([128, 3, F], f32)
        nc.sync.dma_start(posT[:], pos_ap.transpose([1, 0, 2]))
        r = posT[:, 0, :]; th = posT[:, 1, :]; ph = posT[:, 2, :]
        wts = const.tile([128, 256], f32)
        nc.sync.dma_start(wts[:], wts_ap)
        ident = const.tile([128, 128], f32)
        nc.sync.dma_start(ident[:], ident_ap)
        wtsr = const.tile([128, 256], f32r)
        nc.scalar.copy(wtsr[:], wts[:])

        # psi point-major, one tile per segment: PMs[i][p, g-g0, bb]
        PMs = []
        for i, (g0, g1, _) in enumerate(SEGS):
            pm = feat.tile([128, g1 - g0, 32], f32, tag=f"PM{i}")
            nc.vector.memset(pm[:, :, NB:32], 0.0)
            PMs.append(pm)

        act = nc.scalar.activation
        stt = nc.vector.scalar_tensor_tensor
        tt = nc.vector.tensor_tensor
        ts = nc.vector.tensor_scalar
        gtt = nc.gpsimd.tensor_tensor

        # ---- seeds (ACT: Sin set, then Exp set; squares on DVE) ----
        sth = ft("sth"); u = ft("u"); s1 = ft("s1"); sh = ft("sh")
        act(sth[:], th[:], AF.Sin)                          # sin(theta)
        act(u[:], th[:], AF.Sin, scale=-1.0, bias=PI / 2)   # cos(theta)
        act(s1[:], ph[:], AF.Sin, scale=-1.0, bias=PI)      # sin(phi)
        act(sh[:], ph[:], AF.Sin, scale=0.5)                # sin(phi/2)
        E2 = ft("E2"); E3 = ft("E3"); E4 = ft("E4")
        act(E2[:], r[:], AF.Exp, scale=-0.5)
        act(E3[:], r[:], AF.Exp, scale=-1.0 / 3.0)
        act(E4[:], r[:], AF.Exp, scale=-0.25)
        shsq = ft("shsq"); u2 = ft("u2"); stsq = ft("stsq")
        tt(shsq[:], sh[:], sh[:], Alu.mult)
        tt(u2[:], u[:], u[:], Alu.mult)
        tt(stsq[:], sth[:], sth[:], Alu.mult)

        # ---- trig ladders / angular (late-consumer ops on GPSIMD) ----
        c1 = ft("c1")
        ts(c1[:], shsq[:], -2.0, 1.0, Alu.mult, Alu.add)    # cos(phi)
        c1sq = ft("c1sq")
        tt(c1sq[:], c1[:], c1[:], Alu.mult)
        c2t = ft("c2t")
        ts(c2t[:], c1sq[:], -0.5, None, Alu.add)            # cos(2phi)/2
        s2t = ft("s2t")
        tt(s2t[:], s1[:], c1[:], Alu.mult)                  # sin(2phi)/2
        c3t = ft("c3t")
        stt(c3t[:], c1sq[:], 0.75, c1[:], Alu.subtract, Alu.mult)   # cos3/4
        s3t = ft("s3t")
        stt(s3t[:], c1sq[:], 0.25, s1[:], Alu.subtract, Alu.mult)   # sin3/4
        p20 = ft("p20")
        ts(p20[:], u2[:], -1.0 / 3.0, None, Alu.add)
        p30 = ft("p30")
        stt(p30[:], u2[:], 0.6, u[:], Alu.subtract, Alu.mult)
        p33 = ft("p33")
        gtt(p33[:], sth[:], stsq[:], Alu.mult)              # sin^3(theta)
        A1c = ft("A1c"); A1s = ft("A1s")
        tt(A1c[:], sth[:], c1[:], Alu.mult)
        tt(A1s[:], sth[:], s1[:], Alu.mult)
        A2c1 = ft("A2c1"); A2s1 = ft("A2s1")
        tt(A2c1[:], u[:], A1c[:], Alu.mult)
        tt(A2s1[:], u[:], A1s[:], Alu.mult)
        A2c2 = ft("A2c2"); A2s2 = ft("A2s2")
        tt(A2c2[:], stsq[:], c2t[:], Alu.mult)
        tt(A2s2[:], stsq[:], s2t[:], Alu.mult)
        A3c1 = ft("A3c1"); A3s1 = ft("A3s1")
        stt(A3c1[:], u2[:], 0.2, A1c[:], Alu.subtract, Alu.mult)
        stt(A3s1[:], u2[:], 0.2, A1s[:], Alu.subtract, Alu.mult)
        A3c2 = ft("A3c2"); A3s2 = ft("A3s2")
        gtt(A3c2[:], u[:], A2c2[:], Alu.mult)
        gtt(A3s2[:], u[:], A2s2[:], Alu.mult)
        A3c3 = ft("A3c3"); A3s3 = ft("A3s3")
        gtt(A3c3[:], p33[:], c3t[:], Alu.mult)
        gtt(A3s3[:], p33[:], s3t[:], Alu.mult)

        # ---- radial (TT sub-chain on GPSIMD, stt stays on DVE) ----
        E2r = ft("E2r"); E3r = ft("E3r"); E4r = ft("E4r")
        gtt(E2r[:], E2[:], r[:], Alu.mult)
        gtt(E3r[:], E3[:], r[:], Alu.mult)
        gtt(E4r[:], E4[:], r[:], Alu.mult)
        R31 = ft("R31")
        stt(R31[:], r[:], ROOTS31[0], E3r[:], Alu.subtract, Alu.mult)
        R32 = ft("R32")
        gtt(R32[:], E3r[:], r[:], Alu.mult)
        E4r2 = ft("E4r2")
        gtt(E4r2[:], E4r[:], r[:], Alu.mult)
        R41a = ft("R41a"); R41 = ft("R41")
        stt(R41a[:], r[:], ROOTS41[0], E4r[:], Alu.subtract, Alu.mult)
        stt(R41[:], r[:], ROOTS41[1], R41a[:], Alu.subtract, Alu.mult)
        R42 = ft("R42")
        stt(R42[:], r[:], ROOTS42[0], E4r2[:], Alu.subtract, Alu.mult)
        R43 = ft("R43")
        gtt(R43[:], E4r2[:], r[:], Alu.mult)
        t35 = ft("t35")
        stt(t35[:], r[:], ROOTS30[0], E3[:], Alu.subtract, Alu.mult)
        t41 = ft("t41"); t42 = ft("t42")
        stt(t41[:], r[:], ROOTS40[0], E4[:], Alu.subtract, Alu.mult)
        stt(t42[:], r[:], ROOTS40[1], t41[:], Alu.subtract, Alu.mult)

        # ---- pipelined: psi seg-products, transposes, matmuls, DMA ----
        psiT = feat.tile([128, GPTS], f32r)
        out3 = out_ap.rearrange("o (j p) -> o j p", j=NGRP)

        def psi_seg(si):
            g0, g1, _ = SEGS[si]
            PM = PMs[si]
            sl = slice(g0, g1)

            def pslot(b):
                return PM[:, :, b]

            act(pslot(0), r[:, sl], AF.Exp, scale=-1.0)
            stt(pslot(1), r[:, sl], ROOTS20[0], E2[:, sl],
                Alu.subtract, Alu.mult)
            tt(pslot(2), E2r[:, sl], A1s[:, sl], Alu.mult)
            tt(pslot(3), E2r[:, sl], u[:, sl], Alu.mult)
            tt(pslot(4), E2r[:, sl], A1c[:, sl], Alu.mult)
            stt(pslot(5), r[:, sl], ROOTS30[1], t35[:, sl],
                Alu.subtract, Alu.mult)
            tt(pslot(6), R31[:, sl], A1s[:, sl], Alu.mult)
            tt(pslot(7), R31[:, sl], u[:, sl], Alu.mult)
            tt(pslot(8), R31[:, sl], A1c[:, sl], Alu.mult)
            gtt(pslot(9), R32[:, sl], A2s2[:, sl], Alu.mult)
            tt(pslot(10), R32[:, sl], A2s1[:, sl], Alu.mult)
            tt(pslot(11), R32[:, sl], p20[:, sl], Alu.mult)
            tt(pslot(12), R32[:, sl], A2c1[:, sl], Alu.mult)
            gtt(pslot(13), R32[:, sl], A2c2[:, sl], Alu.mult)
            stt(pslot(14), r[:, sl], ROOTS40[2], t42[:, sl],
                Alu.subtract, Alu.mult)
            tt(pslot(15), R41[:, sl], A1s[:, sl], Alu.mult)
            tt(pslot(16), R41[:, sl], u[:, sl], Alu.mult)
            tt(pslot(17), R41[:, sl], A1c[:, sl], Alu.mult)
            gtt(pslot(18), R42[:, sl], A2s2[:, sl], Alu.mult)
            tt(pslot(19), R42[:, sl], A2s1[:, sl], Alu.mult)
            tt(pslot(20), R42[:, sl], p20[:, sl], Alu.mult)
            tt(pslot(21), R42[:, sl], A2c1[:, sl], Alu.mult)
            gtt(pslot(22), R42[:, sl], A2c2[:, sl], Alu.mult)
            gtt(pslot(23), R43[:, sl], A3s3[:, sl], Alu.mult)
            gtt(pslot(24), R43[:, sl], A3s2[:, sl], Alu.mult)
            tt(pslot(25), R43[:, sl], A3s1[:, sl], Alu.mult)
            tt(pslot(26), R43[:, sl], p30[:, sl], Alu.mult)
            tt(pslot(27), R43[:, sl], A3c1[:, sl], Alu.mult)
            gtt(pslot(28), R43[:, sl], A3c2[:, sl], Alu.mult)
            gtt(pslot(29), R43[:, sl], A3c3[:, sl], Alu.mult)

        def transposes_seg(si):
            g0, g1, _ = SEGS[si]
            PM = PMs[si]
            c_lo, c_hi = g0 // 4, g1 // 4
            for cb in range(c_lo, c_hi, 4):
                nb4 = min(4, c_hi - cb)
                tp = pT.tile([128, 512], f32, tag="tp")
                for ci in range(nb4):
                    c = cb + ci
                    nc.tensor.transpose(
                        tp[:, ci * 128:(ci + 1) * 128],
                        PM[:, 4 * c - g0:4 * c - g0 + 4, :], ident[:])
                dst = psiT[:, cb * 128:(cb + nb4) * 128]
                if (cb // 4) % 2 == 0:
                    nc.scalar.copy(dst, tp[:, :nb4 * 128])
                else:
                    nc.vector.tensor_copy(dst, tp[:, :nb4 * 128])

        evac_state = [0]

        def mm_chunk(t, dve_evac_mod):
            n = min(512, GPTS - t * 512)
            for h in range(2):
                so = stg.tile([128, 2048], bf16, tag="so")
                so4 = so.rearrange("p (j q) -> p j q", q=512)
                for jp in (0, 2):
                    ps = pM.tile([128, 1024], f32, tag="ps")
                    for jj in (0, 1):
                        j = jp + jj
                        lhsT = wtsr[32 * j:32 * j + NB,
                                    128 * h:128 * (h + 1)]
                        rhs = psiT[32 * j:32 * j + NB, t * 512:t * 512 + n]
                        nc.tensor.matmul(ps[:, jj * 512:jj * 512 + n],
                                         lhsT, rhs, start=True, stop=True,
                                         tile_position=(32 * j, 0))
                    # one wide copy covers both j outputs; the [n:512] gap
                    # is never DMA'd so copying it is harmless
                    w = 512 + n
                    i = evac_state[0]
                    evac_state[0] += 1
                    if i % 3 == 0:
                        nc.vector.tensor_copy(
                            so[:, jp * 512:jp * 512 + w], ps[:, :w])
                    else:
                        nc.scalar.copy(
                            so[:, jp * 512:jp * 512 + w], ps[:, :w])
                dst = out3[128 * h:128 * (h + 1), :, t * 512:t * 512 + n]
                nc.sync.dma_start(dst, so4[:, :, :n])

        # emission order = scheduler priority: psi products of the next
        # segment outrank evacuations of the previous one on DVE
        psi_seg(0)
        transposes_seg(0)
        psi_seg(1)
        mm_chunk(0, 0)          # early evacs: ACT only
        transposes_seg(1)
        psi_seg(2)
        mm_chunk(1, 0)
        mm_chunk(2, 0)
        transposes_seg(2)
        for t in (3, 4, 5, 6):
            mm_chunk(t, 2)      # late evacs: alternate DVE/ACT


def _get_program():
    if "nc" not in _PROGRAM_CACHE:
        _PROGRAM_CACHE["nc"] = _build_program()
    return _PROGRAM_CACHE["nc"]


# ---------------------------------------------------------------- host wrapper
def _host_prep(position, coeffs):
    K = _fold_constants()
    Cs = (np.asarray(coeffs, np.float64).reshape(OUTC * INC, NB)
          * K[None, :]).astype(np.float32)
    W = np.zeros((128, 256), np.float32)
    for j in range(NGRP):
        W[32 * j:32 * j + NB, :] = Cs.T
    pts = np.asarray(position, np.float32).reshape(PTS, 3)
    pos_cores = []
    for k in range(NCORES):
        sl = pts[k * CPTS:(k + 1) * CPTS]
        v = sl.reshape(NGRP, NROUND, 128, 3)       # [j, c, p, coord]
        v = np.transpose(v, (3, 2, 1, 0))          # [coord, p, c, j]
        pos_cores.append(np.ascontiguousarray(v.reshape(3, 128, NROUND * NGRP)))
    return pos_cores, W


def kernel(position, coeffs, _collect=None):
    from concourse.bass_utils import run_bass_kernel_spmd

    pos_cores, W = _host_prep(position, coeffs)
    ident = np.eye(128, dtype=np.float32)
    in_maps = [{"pos": pos_cores[k], "wts": W, "ident": ident}
               for k in range(NCORES)]
    nc = _get_program()
    try:
        res = run_bass_kernel_spmd(nc, in_maps, core_ids=list(range(NCORES)))
    except Exception:
        # transient NRT/axon failures (e.g. a wedged core from a previous
        # process) usually clear on retry
        res = run_bass_kernel_spmd(nc, in_maps, core_ids=list(range(NCORES)))
    if _collect is not None:
        _collect.append(res)
    full = np.concatenate(
        [np.asarray(res.results[k]["out"]).astype(np.float32)
         for k in range(NCORES)], axis=1)
    return full.reshape(OUTC, INC, OUTN, CONV_N)
